# revision 1
# baseline (speedup 1.0000x reference)
"""Trainium2 Bass kernel for the Tsit5 Neural-ODE problem.

Math (per RK micro-step, 64 steps, autonomous MLP dynamics):
    k_j = f(A_j),  A_1 = y,  A_j = y + sum_{i<j} dt*a_ji k_i
    f(x) = W3.T tanh-stack ... computed in TRANSPOSED layout [feat, batch]
so that D=128 features sit exactly on the 128 SBUF partitions and the
MLP weights are the stationary matmul operands with no transposes.

Sharding: data-parallel over batch: 8192 rows -> 1024 per core x 8 cores,
each core runs 2 independent half-streams of 512 to pipeline PE/ACT/DVE.
Matmuls run in float32r (full PE rate, ~1.5e-4 rel err vs fp32).

b3 never materializes: A_j = A'_j + s_j*b3 with s_j = dt*sum_i a_ji, and
W1.T A_j = W1.T A'_j + s_j (W1.T b3), which folds into a per-stage L1 bias.
"""

import numpy as np

import concourse.bass as bass
import concourse.mybir as mybir
import concourse.tile as tile
from concourse import bacc
from concourse.bass_utils import run_bass_kernel_spmd

f32 = mybir.dt.float32
f32r = mybir.dt.float32r

NCORES = 8
B, D, H = 8192, 128, 512
NSTEPS = 64
BC = B // NCORES          # batch per core
NHALF = 2                 # independent streams per core
BH = BC // NHALF          # 512 batch per stream
MCH = H // 128            # 4 H-chunks

# Tsit5 tableau (Tsitouras 2011)
A_TAB = {
    2: [0.161],
    3: [-0.008480655492356989, 0.335480655492357],
    4: [2.8971530571054935, -6.359448489975075, 4.3622954328695815],
    5: [5.325864828439257, -11.748883564062828, 7.4955393428898365,
        -0.09249506636175525],
    6: [5.86145544294642, -12.92096931784711, 8.159367898576159,
        -0.071584973281401, -0.028269050394068383],
}
B_TAB = [0.09646076681806523, 0.01, 0.4798896504144996, 1.379008574103742,
         -3.290069515436081, 2.324710524099774]

Tanh = mybir.ActivationFunctionType.Tanh
MULT = mybir.AluOpType.mult
ADD = mybir.AluOpType.add

_PROGRAM_CACHE = {}


def _build_program(dt, zero_bias, nsteps):
    """Emit the Bass/Tile program for one core (shard of BC rows)."""
    c = {l: [dt * a for a in A_TAB[l]] for l in A_TAB}   # c[l][j-1], j=1..l-1
    by = [dt * b for b in B_TAB]

    nc = bacc.Bacc("TRN2", target_bir_lowering=False, debug=False)
    y0t_d = nc.dram_tensor("y0t", [D, BC], f32, kind="ExternalInput").ap()
    w1_d = nc.dram_tensor("w1", [D, H], f32r, kind="ExternalInput").ap()
    w2_d = nc.dram_tensor("w2", [H, H], f32r, kind="ExternalInput").ap()
    w3_d = nc.dram_tensor("w3", [H, D], f32r, kind="ExternalInput").ap()
    if not zero_bias:
        # b1s: per (stage, m-chunk) L1 bias cols (24), b2s: 4 cols, s7b3: 1
        b1s_d = nc.dram_tensor("b1s", [128, 6 * MCH], f32,
                               kind="ExternalInput").ap()
        b2s_d = nc.dram_tensor("b2s", [128, MCH], f32,
                               kind="ExternalInput").ap()
        s7b3_d = nc.dram_tensor("s7b3", [128, 1], f32,
                                kind="ExternalInput").ap()
    yt_d = nc.dram_tensor("yt", [D, BC], f32, kind="ExternalOutput").ap()

    with tile.TileContext(nc) as tc:
        with (
            tc.tile_pool(name="const", bufs=1) as const,
            tc.tile_pool(name="state", bufs=2) as state,
            tc.tile_pool(name="work", bufs=2) as work,
            tc.tile_pool(name="psA", bufs=2, space="PSUM") as psA,
            tc.tile_pool(name="psB", bufs=2, space="PSUM") as psB,
        ):
            ps_pools = [psA, psB]
            # ---- load constants ----
            w1t = const.tile([D, H], f32r, tag="w1")
            nc.sync.dma_start(w1t[:], w1_d)
            w2t = []
            for k in range(MCH):
                t = const.tile([128, H], f32r, tag=f"w2_{k}")
                nc.sync.dma_start(t[:], w2_d[k * 128:(k + 1) * 128, :])
                w2t.append(t)
            w3t = []
            for k in range(MCH):
                t = const.tile([128, D], f32r, tag=f"w3_{k}")
                nc.sync.dma_start(t[:], w3_d[k * 128:(k + 1) * 128, :])
                w3t.append(t)
            if not zero_bias:
                b1st = const.tile([128, 6 * MCH], f32, tag="b1s")
                nc.sync.dma_start(b1st[:], b1s_d)
                b2st = const.tile([128, MCH], f32, tag="b2s")
                nc.sync.dma_start(b2st[:], b2s_d)
                s7b3t = const.tile([128, 1], f32, tag="s7b3")
                nc.sync.dma_start(s7b3t[:], s7b3_d)

            # ---- per-half state ----
            halves = []
            for h in range(NHALF):
                y = state.tile([D, BH], f32, tag=f"y{h}")
                nc.sync.dma_start(y[:], y0t_d[:, h * BH:(h + 1) * BH])
                yr = state.tile([D, BH], f32r, tag=f"yr{h}")
                nc.vector.tensor_copy(yr[:], y[:])
                halves.append({"y": y, "yr": yr, "h": h})

            def l1_bias(i, m):
                if zero_bias:
                    return 0.0
                return b1st[:, (i - 1) * MCH + m:(i - 1) * MCH + m + 1]

            def l2_bias(m):
                if zero_bias:
                    return 0.0
                return b2st[:, m:m + 1]

            def tanh_pair(ps_tile, out_tile, g, bias_fn):
                """tanh over a [128,1024] psum concat tile (m=2g, 2g+1)."""
                b0 = bias_fn(2 * g)
                b1 = bias_fn(2 * g + 1)
                if isinstance(b0, float) and isinstance(b1, float):
                    nc.scalar.activation(out_tile[:], ps_tile[:], Tanh)
                else:
                    nc.scalar.activation(
                        out_tile[:, 0:BH], ps_tile[:, 0:BH], Tanh, bias=b0)
                    nc.scalar.activation(
                        out_tile[:, BH:2 * BH], ps_tile[:, BH:2 * BH], Tanh,
                        bias=b1)

            def emit_stage(hs, i, step, last_step):
                h = hs["h"]
                ps = ps_pools[h]
                a = hs["a"] if i > 1 else hs["yr"]
                # --- L1: z = W1.T a  (4 m-chunks as 2 concat psum tiles) ---
                h1 = []
                for g in range(2):
                    z1 = ps.tile([128, 2 * BH], f32, tag=f"ps{h}")
                    for mm in range(2):
                        m = 2 * g + mm
                        nc.tensor.matmul(
                            z1[:, mm * BH:(mm + 1) * BH],
                            w1t[:, m * 128:(m + 1) * 128], a[:],
                            start=True, stop=True)
                    t = work.tile([128, 2 * BH], f32r, tag=f"h1_{h}_{g}")
                    tanh_pair(z1, t, g, lambda m: l1_bias(i, m))
                    h1.append(t)
                # --- L2: z = W2.T h1 (accumulate over 4 k-chunks) ---
                h2 = []
                for g in range(2):
                    z2 = ps.tile([128, 2 * BH], f32, tag=f"ps{h}")
                    for mm in range(2):
                        m = 2 * g + mm
                        for k in range(MCH):
                            nc.tensor.matmul(
                                z2[:, mm * BH:(mm + 1) * BH],
                                w2t[k][:, m * 128:(m + 1) * 128],
                                h1[k // 2][:, (k % 2) * BH:(k % 2 + 1) * BH],
                                start=(k == 0), stop=(k == MCH - 1))
                    t = work.tile([128, 2 * BH], f32r, tag=f"h2_{h}_{g}")
                    tanh_pair(z2, t, g, l2_bias)
                    h2.append(t)
                # --- L3: u = W3.T h2 (accumulate over 4 k-chunks) ---
                u = ps.tile([128, BH], f32, tag=f"ps{h}")
                for k in range(MCH):
                    nc.tensor.matmul(
                        u[:], w3t[k],
                        h2[k // 2][:, (k % 2) * BH:(k % 2 + 1) * BH],
                        start=(k == 0), stop=(k == MCH - 1))

                # --- RK combinations consuming u ---
                accs = hs["accs"]

                def src(l):
                    t = accs.get(l)
                    return hs["y"] if t is None else t

                # critical path first: finalize next stage argument (f32r)
                if i < 6:
                    a_next = work.tile([D, BH], f32r, tag=f"a{h}")
                    nc.vector.scalar_tensor_tensor(
                        a_next[:], u[:], c[i + 1][i - 1], src(i + 1)[:],
                        MULT, ADD)
                    hs["a"] = a_next
                # park u in SBUF so its PSUM bank frees early
                uc = work.tile([128, BH], f32, tag=f"u{h}")
                nc.vector.tensor_copy(uc[:], u[:])
                for l in range(i + 2, 7):
                    acc = work.tile([D, BH], f32, tag=f"acc{h}_{l}")
                    nc.vector.scalar_tensor_tensor(
                        acc[:], uc[:], c[l][i - 1], src(l)[:], MULT, ADD)
                    accs[l] = acc
                # y accumulator
                if i < 6:
                    acc = work.tile([D, BH], f32, tag=f"acc{h}_y")
                    nc.vector.scalar_tensor_tensor(
                        acc[:], uc[:], by[i - 1], src("y")[:], MULT, ADD)
                    accs["y"] = acc
                else:
                    y_new = state.tile([D, BH], f32, tag=f"y{h}")
                    if zero_bias:
                        nc.vector.scalar_tensor_tensor(
                            y_new[:], uc[:], by[5], src("y")[:], MULT, ADD)
                    else:
                        tmp = work.tile([D, BH], f32, tag=f"acc{h}_y")
                        nc.vector.scalar_tensor_tensor(
                            tmp[:], uc[:], by[5], src("y")[:], MULT, ADD)
                        nc.vector.tensor_scalar(
                            y_new[:], tmp[:], s7b3t[:, 0:1], None, ADD)
                    hs["y"] = y_new
                    if not last_step:
                        yr = state.tile([D, BH], f32r, tag=f"yr{h}")
                        nc.vector.tensor_copy(yr[:], y_new[:])
                        hs["yr"] = yr

            for step in range(nsteps):
                for hs in halves:
                    hs["accs"] = {}
                for i in range(1, 7):
                    for hs in halves:
                        emit_stage(hs, i, step, step == nsteps - 1)

            for hs in halves:
                h = hs["h"]
                nc.sync.dma_start(yt_d[:, h * BH:(h + 1) * BH], hs["y"][:])

    nc.compile()
    return nc


def _get_program(dt, zero_bias, nsteps=NSTEPS):
    key = (round(float(dt), 12), bool(zero_bias), int(nsteps))
    if key not in _PROGRAM_CACHE:
        _PROGRAM_CACHE[key] = _build_program(dt, zero_bias, nsteps)
    return _PROGRAM_CACHE[key]


def _prep_inputs(y0, ts, W1, b1, W2, b2, W3, b3, nsteps=NSTEPS):
    y0 = np.asarray(y0, dtype=np.float32)
    ts = np.asarray(ts, dtype=np.float32)
    W1 = np.ascontiguousarray(np.asarray(W1, dtype=np.float32))
    W2 = np.ascontiguousarray(np.asarray(W2, dtype=np.float32))
    W3 = np.ascontiguousarray(np.asarray(W3, dtype=np.float32))
    b1 = np.asarray(b1, dtype=np.float32)
    b2 = np.asarray(b2, dtype=np.float32)
    b3 = np.asarray(b3, dtype=np.float32)
    dt = float(ts[-1] - ts[0]) / nsteps
    zero_bias = not (np.any(b1) or np.any(b2) or np.any(b3))

    in_maps = []
    for core in range(NCORES):
        shard = np.ascontiguousarray(
            y0[core * BC:(core + 1) * BC, :].T)  # [D, BC]
        m = {"y0t": shard, "w1": W1, "w2": W2, "w3": W3}
        if not zero_bias:
            # L1 bias per stage i: b1 + s_i * (W1.T b3); stage 1 has s_1 = 0
            w1tb3 = W1.T @ b3  # [H]
            cols = []
            for i in range(1, 7):
                s_i = dt * sum(A_TAB[i]) if i > 1 else 0.0
                bi = b1 + s_i * w1tb3
                for mm in range(MCH):
                    cols.append(bi[mm * 128:(mm + 1) * 128])
            m["b1s"] = np.ascontiguousarray(np.stack(cols, axis=1),
                                            dtype=np.float32)
            m["b2s"] = np.ascontiguousarray(
                np.stack([b2[mm * 128:(mm + 1) * 128] for mm in range(MCH)],
                         axis=1), dtype=np.float32)
            s7 = dt * sum(B_TAB)
            m["s7b3"] = np.ascontiguousarray(
                (s7 * b3).reshape(128, 1), dtype=np.float32)
        in_maps.append(m)
    return dt, zero_bias, in_maps


def _run(inputs, trace=False, nsteps=NSTEPS):
    dt, zero_bias, in_maps = _prep_inputs(**inputs, nsteps=nsteps)
    nc = _get_program(dt, zero_bias, nsteps)
    res = run_bass_kernel_spmd(nc, in_maps, list(range(NCORES)), trace=trace)
    out = np.empty((B, D), dtype=np.float32)
    for core in range(NCORES):
        out[core * BC:(core + 1) * BC, :] = res.results[core]["yt"].T
    return out, res


def kernel(**inputs) -> np.ndarray:
    out, _ = _run(inputs, trace=False)
    return out


# revision 7
# speedup vs baseline: 17.4588x; 17.4588x over previous
"""Trainium2 Bass kernel for the Tsit5 Neural-ODE problem.

Math (per RK micro-step, 64 steps, autonomous MLP dynamics):
    k_j = f(A_j),  A_1 = y,  A_j = y + sum_{i<j} dt*a_ji k_i
    f(x) = W3.T tanh-stack ... computed in TRANSPOSED layout [feat, batch]
so that D=128 features sit exactly on the 128 SBUF partitions and the
MLP weights are the stationary matmul operands with no transposes.

Sharding: data-parallel over batch: 8192 rows -> 1024 per core x 8 cores,
each core runs 2 independent half-streams of 512 to pipeline PE/ACT/DVE.
Matmuls run in float32r (full PE rate, ~1.5e-4 rel err vs fp32).

b3 never materializes: A_j = A'_j + s_j*b3 with s_j = dt*sum_i a_ji, and
W1.T A_j = W1.T A'_j + s_j (W1.T b3), which folds into a per-stage L1 bias.
"""

import numpy as np

import concourse.bass as bass
import concourse.mybir as mybir
import concourse.tile as tile
from concourse import bacc
from concourse.bass_utils import run_bass_kernel_spmd

f32 = mybir.dt.float32
f32r = mybir.dt.float32r

NCORES = 8
B, D, H = 8192, 128, 512
NSTEPS = 64
BC = B // NCORES          # batch per core
NHALF = 2                 # independent streams per core
BH = BC // NHALF          # 512 batch per stream
MCH = H // 128            # 4 H-chunks

# Tsit5 tableau (Tsitouras 2011)
A_TAB = {
    2: [0.161],
    3: [-0.008480655492356989, 0.335480655492357],
    4: [2.8971530571054935, -6.359448489975075, 4.3622954328695815],
    5: [5.325864828439257, -11.748883564062828, 7.4955393428898365,
        -0.09249506636175525],
    6: [5.86145544294642, -12.92096931784711, 8.159367898576159,
        -0.071584973281401, -0.028269050394068383],
}
B_TAB = [0.09646076681806523, 0.01, 0.4798896504144996, 1.379008574103742,
         -3.290069515436081, 2.324710524099774]

Tanh = mybir.ActivationFunctionType.Tanh
MULT = mybir.AluOpType.mult
ADD = mybir.AluOpType.add

_PROGRAM_CACHE = {}

# "wide": one [128, 2*BH] psum tile per layer-half (z1 g0/g1), tanh per pair.
# "full": one [128, 4*BH] psum tile per layer (all 4 m-chunks), single tanh.
PSUM_MODE = "wide"
# emission-priority stage offset between the two half-streams
STAGE_LAG = 0


def _build_program(dt, zero_bias, nsteps, psum_mode=None):
    """Emit the Bass/Tile program for one core (shard of BC rows)."""
    if psum_mode is None:
        psum_mode = PSUM_MODE
    c = {l: [dt * a for a in A_TAB[l]] for l in A_TAB}   # c[l][j-1], j=1..l-1
    by = [dt * b for b in B_TAB]

    nc = bacc.Bacc("TRN2", target_bir_lowering=False, debug=False)
    y0t_d = nc.dram_tensor("y0t", [D, BC], f32, kind="ExternalInput").ap()
    w1_d = nc.dram_tensor("w1", [D, H], f32r, kind="ExternalInput").ap()
    w2_d = nc.dram_tensor("w2", [H, H], f32r, kind="ExternalInput").ap()
    w3_d = nc.dram_tensor("w3", [H, D], f32r, kind="ExternalInput").ap()
    if not zero_bias:
        # b1s: per (stage, m-chunk) L1 bias cols (24), b2s: 4 cols, s7b3: 1
        b1s_d = nc.dram_tensor("b1s", [128, 6 * MCH], f32,
                               kind="ExternalInput").ap()
        b2s_d = nc.dram_tensor("b2s", [128, MCH], f32,
                               kind="ExternalInput").ap()
        s7b3_d = nc.dram_tensor("s7b3", [128, 1], f32,
                                kind="ExternalInput").ap()
    yt_d = nc.dram_tensor("yt", [D, BC], f32, kind="ExternalOutput").ap()

    with tile.TileContext(nc) as tc:
        with (
            tc.tile_pool(name="const", bufs=1) as const,
            tc.tile_pool(name="state", bufs=2) as state,
            tc.tile_pool(name="work", bufs=2) as work,
            tc.tile_pool(name="psA", bufs=2 if psum_mode == "wide" else 1,
                         space="PSUM") as psA,
            tc.tile_pool(name="psB", bufs=2 if psum_mode == "wide" else 1,
                         space="PSUM") as psB,
        ):
            ps_pools = [psA, psB]
            # ---- load constants ----
            w1t = const.tile([D, H], f32r, tag="w1")
            nc.sync.dma_start(w1t[:], w1_d)
            w2t = []
            for k in range(MCH):
                t = const.tile([128, H], f32r, tag=f"w2_{k}")
                nc.sync.dma_start(t[:], w2_d[k * 128:(k + 1) * 128, :])
                w2t.append(t)
            w3t = []
            for k in range(MCH):
                t = const.tile([128, D], f32r, tag=f"w3_{k}")
                nc.sync.dma_start(t[:], w3_d[k * 128:(k + 1) * 128, :])
                w3t.append(t)
            if not zero_bias:
                b1st = const.tile([128, 6 * MCH], f32, tag="b1s")
                nc.sync.dma_start(b1st[:], b1s_d)
                b2st = const.tile([128, MCH], f32, tag="b2s")
                nc.sync.dma_start(b2st[:], b2s_d)
                s7b3t = const.tile([128, 1], f32, tag="s7b3")
                nc.sync.dma_start(s7b3t[:], s7b3_d)

            # ---- per-half state ----
            halves = []
            for h in range(NHALF):
                y = state.tile([D, BH], f32, tag=f"y{h}")
                nc.sync.dma_start(y[:], y0t_d[:, h * BH:(h + 1) * BH])
                yr = state.tile([D, BH], f32r, tag=f"yr{h}")
                nc.vector.tensor_copy(yr[:], y[:])
                halves.append({"y": y, "yr": yr, "h": h})

            def l1_bias(i, m):
                if zero_bias:
                    return 0.0
                return b1st[:, (i - 1) * MCH + m:(i - 1) * MCH + m + 1]

            def l2_bias(m):
                if zero_bias:
                    return 0.0
                return b2st[:, m:m + 1]

            # group width in m-chunks per psum tile: "wide"=2, "full"=4
            GW = 2 if psum_mode == "wide" else 4
            NG = MCH // GW

            def tanh_group(ps_tile, out_tile, g, bias_fn):
                """tanh over a [128, GW*BH] psum concat tile (m=GW*g..)."""
                biases = [bias_fn(GW * g + mm) for mm in range(GW)]
                if all(isinstance(b, float) for b in biases):
                    nc.scalar.activation(out_tile[:], ps_tile[:], Tanh)
                else:
                    for mm, b in enumerate(biases):
                        nc.scalar.activation(
                            out_tile[:, mm * BH:(mm + 1) * BH],
                            ps_tile[:, mm * BH:(mm + 1) * BH], Tanh, bias=b)

            def emit_stage(hs, i, step, last_step):
                h = hs["h"]
                ps = ps_pools[h]
                a = hs["a"] if i > 1 else hs["yr"]
                # --- L1: z = W1.T a  (4 m-chunks as NG concat psum tiles) ---
                h1 = []
                for g in range(NG):
                    z1 = ps.tile([128, GW * BH], f32, tag=f"ps{h}", name="z1")
                    for mm in range(GW):
                        m = GW * g + mm
                        nc.tensor.matmul(
                            z1[:, mm * BH:(mm + 1) * BH],
                            w1t[:, m * 128:(m + 1) * 128], a[:],
                            start=True, stop=True)
                    t = work.tile([128, GW * BH], f32r, tag=f"h1_{h}_{g}",
                                  name="h1")
                    tanh_group(z1, t, g, lambda m: l1_bias(i, m))
                    h1.append(t)
                # --- L2: z = W2.T h1 (accumulate over 4 k-chunks) ---
                h2 = []
                for g in range(NG):
                    z2 = ps.tile([128, GW * BH], f32, tag=f"ps{h}", name="z2")
                    for mm in range(GW):
                        m = GW * g + mm
                        for k in range(MCH):
                            nc.tensor.matmul(
                                z2[:, mm * BH:(mm + 1) * BH],
                                w2t[k][:, m * 128:(m + 1) * 128],
                                h1[k // GW][:, (k % GW) * BH:
                                            (k % GW + 1) * BH],
                                start=(k == 0), stop=(k == MCH - 1))
                    t = work.tile([128, GW * BH], f32r, tag=f"h2_{h}_{g}",
                                  name="h2")
                    tanh_group(z2, t, g, l2_bias)
                    h2.append(t)
                # --- L3: u = W3.T h2 (accumulate over 4 k-chunks) ---
                u = ps.tile([128, BH], f32, tag=f"ps{h}", name="u")
                for k in range(MCH):
                    nc.tensor.matmul(
                        u[:], w3t[k],
                        h2[k // GW][:, (k % GW) * BH:(k % GW + 1) * BH],
                        start=(k == 0), stop=(k == MCH - 1))

                # --- RK combinations consuming u ---
                accs = hs["accs"]

                def src(l):
                    t = accs.get(l)
                    return hs["y"] if t is None else t

                # critical path first: finalize next stage argument (f32r)
                if i < 6:
                    a_next = work.tile([D, BH], f32r, tag=f"a{h}")
                    nc.vector.scalar_tensor_tensor(
                        a_next[:], u[:], c[i + 1][i - 1], src(i + 1)[:],
                        MULT, ADD)
                    hs["a"] = a_next
                # park u in SBUF so its PSUM bank frees early
                uc = work.tile([128, BH], f32, tag=f"u{h}")
                nc.vector.tensor_copy(uc[:], u[:])
                for l in range(i + 2, 7):
                    acc = work.tile([D, BH], f32, tag=f"acc{h}_{l}")
                    nc.vector.scalar_tensor_tensor(
                        acc[:], uc[:], c[l][i - 1], src(l)[:], MULT, ADD)
                    accs[l] = acc
                # y accumulator
                if i < 6:
                    acc = work.tile([D, BH], f32, tag=f"acc{h}_y")
                    nc.vector.scalar_tensor_tensor(
                        acc[:], uc[:], by[i - 1], src("y")[:], MULT, ADD)
                    accs["y"] = acc
                else:
                    y_new = state.tile([D, BH], f32, tag=f"y{h}")
                    if zero_bias:
                        nc.vector.scalar_tensor_tensor(
                            y_new[:], uc[:], by[5], src("y")[:], MULT, ADD)
                    else:
                        tmp = work.tile([D, BH], f32, tag=f"acc{h}_y")
                        nc.vector.scalar_tensor_tensor(
                            tmp[:], uc[:], by[5], src("y")[:], MULT, ADD)
                        nc.vector.tensor_scalar(
                            y_new[:], tmp[:], s7b3t[:, 0:1], None, ADD)
                    hs["y"] = y_new
                    if not last_step:
                        yr = state.tile([D, BH], f32r, tag=f"yr{h}")
                        nc.vector.tensor_copy(yr[:], y_new[:])
                        hs["yr"] = yr

            # emit halves with a stage lag so their engine phases interleave
            total_stages = nsteps * 6
            for t in range(total_stages + STAGE_LAG):
                for hs, t_h in ((halves[0], t), (halves[1], t - STAGE_LAG)):
                    if not (0 <= t_h < total_stages):
                        continue
                    step, i = divmod(t_h, 6)
                    if i == 0:
                        hs["accs"] = {}
                    emit_stage(hs, i + 1, step, step == nsteps - 1)

            for hs in halves:
                h = hs["h"]
                nc.sync.dma_start(yt_d[:, h * BH:(h + 1) * BH], hs["y"][:])

    nc.compile()
    return nc


def _get_program(dt, zero_bias, nsteps=NSTEPS, psum_mode=None):
    key = (round(float(dt), 12), bool(zero_bias), int(nsteps),
           psum_mode or PSUM_MODE)
    if key not in _PROGRAM_CACHE:
        _PROGRAM_CACHE[key] = _build_program(dt, zero_bias, nsteps, psum_mode)
    return _PROGRAM_CACHE[key]


def _prep_inputs(y0, ts, W1, b1, W2, b2, W3, b3, nsteps=NSTEPS):
    y0 = np.asarray(y0, dtype=np.float32)
    ts = np.asarray(ts, dtype=np.float32)
    W1 = np.ascontiguousarray(np.asarray(W1, dtype=np.float32))
    W2 = np.ascontiguousarray(np.asarray(W2, dtype=np.float32))
    W3 = np.ascontiguousarray(np.asarray(W3, dtype=np.float32))
    b1 = np.asarray(b1, dtype=np.float32)
    b2 = np.asarray(b2, dtype=np.float32)
    b3 = np.asarray(b3, dtype=np.float32)
    dt = float(ts[-1] - ts[0]) / nsteps
    zero_bias = not (np.any(b1) or np.any(b2) or np.any(b3))

    in_maps = []
    for core in range(NCORES):
        shard = np.ascontiguousarray(
            y0[core * BC:(core + 1) * BC, :].T)  # [D, BC]
        m = {"y0t": shard, "w1": W1, "w2": W2, "w3": W3}
        if not zero_bias:
            # L1 bias per stage i: b1 + s_i * (W1.T b3); stage 1 has s_1 = 0
            w1tb3 = W1.T @ b3  # [H]
            cols = []
            for i in range(1, 7):
                s_i = dt * sum(A_TAB[i]) if i > 1 else 0.0
                bi = b1 + s_i * w1tb3
                for mm in range(MCH):
                    cols.append(bi[mm * 128:(mm + 1) * 128])
            m["b1s"] = np.ascontiguousarray(np.stack(cols, axis=1),
                                            dtype=np.float32)
            m["b2s"] = np.ascontiguousarray(
                np.stack([b2[mm * 128:(mm + 1) * 128] for mm in range(MCH)],
                         axis=1), dtype=np.float32)
            s7 = dt * sum(B_TAB)
            m["s7b3"] = np.ascontiguousarray(
                (s7 * b3).reshape(128, 1), dtype=np.float32)
        in_maps.append(m)
    return dt, zero_bias, in_maps


def _run(inputs, trace=False, nsteps=NSTEPS):
    dt, zero_bias, in_maps = _prep_inputs(**inputs, nsteps=nsteps)
    nc = _get_program(dt, zero_bias, nsteps)
    res = run_bass_kernel_spmd(nc, in_maps, list(range(NCORES)), trace=trace)
    out = np.empty((B, D), dtype=np.float32)
    for core in range(NCORES):
        out[core * BC:(core + 1) * BC, :] = res.results[core]["yt"].T
    return out, res


def kernel(**inputs) -> np.ndarray:
    out, _ = _run(inputs, trace=False)
    return out
